# revision 7
# baseline (speedup 1.0000x reference)
"""Trainium2 Bass kernel for MultiHeadAttention (B=2, S=4096, D=512, H=8).

Sharding: 16 (batch, head) units across 8 cores -> each core owns one batch
and a contiguous pair of heads (2 heads x 64 depth = 128 columns of the
QKV projections, 128 rows of the output projection).

Key ideas:
  * Mask compression on host: keys with mask==1 receive -1e9 before softmax,
    so their probability is exactly 0 in fp32. We drop those keys entirely
    (gather unmasked rows of x2), roughly halving scores/softmax/AV work.
    Dropped-key handling is exact, not approximate.
  * Scores run out of a transposed activation layout:
      Q_T, K_T: [128(=2 heads x 64 depth), S]  (from x1^T / x2c^T inputs)
    scores for one key-tile land as [128 keys, 1024(=2 heads x 512 queries)]
    in PSUM, and a single ScalarE activation does exp(scores/8) PSUM->SBUF
    into bf16 P tiles.
  * AV uses the "form B" orientation: out[128 queries, 65] accumulated over
    key tiles with the P tile as the (bf16) stationary operand and the
    65-column V^T-with-mask-column tile as the moving operand. Cost-wise the
    moving free size is 65 instead of 512, halving the PE time of AV
    relative to the V^T @ P orientation. Column 64 accumulates the softmax
    denominator (mask column rides along in V).
  * Normalization is a per-query scalar: reciprocal of column 64 and one
    VectorE scalar-mul -> bf16 O block [128 q, 64]. A PE transpose brings it
    to [64, 128 q]; both heads stack into a [128, 128] tile so the output
    projection is a single 128-contraction matmul per 128-row output block
    (both heads reduced in one pass).
  * All fp32 matmul operands are float32r (the PE's fast single-pass fp32
    mode); P/V/O/Wo run in bf16 (the loose 2e-2 tolerance gives >10x
    headroom for bf16's ~0.1% rounding).
  * Output is written in bf16 (halves the store DMA); host sums the 4
    per-core partials of each batch in fp32 and adds bo.

Non-zero q/k/v biases or an all-masked batch fall back to a numpy reference
(those inputs cannot occur with the problem's setup_inputs).
"""

import numpy as np

B, S, D, H = 2, 4096, 512, 8
DH = 64  # depth per head
NCORES = 8

_RUNTIMES = {}


def _build_program(skc: int, reps: int = 1):
    """Build the per-core Bass program. skc = padded compressed key count."""
    import concourse.bacc as bacc
    import concourse.mybir as mybir
    from concourse.masks import make_identity
    from concourse.tile import TileContext

    f32 = mybir.dt.float32
    f32r = mybir.dt.float32r
    bf16 = mybir.dt.bfloat16
    EXP = mybir.ActivationFunctionType.Exp
    r = lambda ap: ap.bitcast(mybir.dt.float32r)  # fast fp32 matmul mode

    NT = skc // 128  # key tiles
    NQC = S // 512  # query chunks (512 wide)
    NKC = (skc + 511) // 512  # key chunks for the K/V projections

    nc = bacc.Bacc("TRN2", target_bir_lowering=False, debug=False, num_devices=NCORES)

    x1t = nc.dram_tensor("x1t", [D, S], bf16, kind="ExternalInput")
    x2ct = nc.dram_tensor("x2ct", [D, skc], bf16, kind="ExternalInput")
    maskb = nc.dram_tensor("maskb", [128, NT], bf16, kind="ExternalInput")
    wq = nc.dram_tensor("wq", [D, 128], bf16, kind="ExternalInput")
    wk = nc.dram_tensor("wk", [D, 128], bf16, kind="ExternalInput")
    wv = nc.dram_tensor("wv", [D, 128], bf16, kind="ExternalInput")
    wo2 = nc.dram_tensor("wo2", [128, 512], f32r, kind="ExternalInput")
    out = nc.dram_tensor("out", [S, D], bf16, kind="ExternalOutput")

    with nc.allow_low_precision(
        reason="bf16 P/V/O tiles; fp32 PSUM accumulation; 2e-2 tolerance"
    ), TileContext(nc) as tc:
        with (
            tc.tile_pool(name="consts", bufs=1) as consts,
            tc.tile_pool(name="bigsb", bufs=1) as bigsb,
            tc.tile_pool(name="xstream", bufs=3) as xstream,
            tc.tile_pool(name="pexp", bufs=34) as pexp,
            tc.tile_pool(name="work", bufs=3) as work,
            tc.tile_pool(name="ps_big", bufs=2, space="PSUM") as ps_big,
            tc.tile_pool(name="ps_acc", bufs=2, space="PSUM") as ps_acc,
            tc.tile_pool(name="ps_misc", bufs=2, space="PSUM") as ps_misc,
        ):
            # ---- constants / persistent buffers (DMA issue order matters:
            # the DMA device drains them in order) ----
            # x1 chunk 0 first, split per k-tile so the first Q matmul can
            # start after only a quarter of the transfer
            x1r = x1t.rearrange("(t p) s -> p t s", p=128)
            wq_sb = consts.tile([128, 4, 128], bf16)
            nc.sync.dma_start(out=wq_sb, in_=wq.rearrange("(t p) m -> p t m", p=128))
            wk_sb = consts.tile([128, 4, 128], bf16)
            nc.sync.dma_start(out=wk_sb, in_=wk.rearrange("(t p) m -> p t m", p=128))
            x1c0 = xstream.tile([128, 4, 512], bf16, tag="xs")
            for kt in range(4):
                nc.sync.dma_start(out=x1c0[:, kt, :], in_=x1r[:, kt, 0:512])
            x2all = bigsb.tile([128, 4, skc], bf16)
            x2r = x2ct.rearrange("(t p) s -> p t s", p=128)
            c0w = min(512, skc)
            c0a = min(128, c0w)  # first key-tile lands fast -> early first score
            nc.sync.dma_start(out=x2all[:, :, 0:c0a], in_=x2r[:, :, 0:c0a])
            wv_sb = consts.tile([128, 4, 128], bf16)
            nc.sync.dma_start(out=wv_sb, in_=wv.rearrange("(t p) m -> p t m", p=128))
            maskb_sb = consts.tile([128, NT], bf16)
            nc.sync.dma_start(out=maskb_sb, in_=maskb[:, :])
            if c0w > c0a:
                nc.sync.dma_start(out=x2all[:, :, c0a:c0w], in_=x2r[:, :, c0a:c0w])
            for c in range(1, NKC):
                cw = min(512, skc - c * 512)
                nc.sync.dma_start(
                    out=x2all[:, :, c * 512 : c * 512 + cw],
                    in_=x2r[:, :, c * 512 : c * 512 + cw],
                )
            wo2_sb = consts.tile([128, 512], f32r)
            nc.sync.dma_start(out=wo2_sb, in_=wo2[:, :])

            ident = consts.tile([128, 128], f32)
            make_identity(nc, ident)

            # ---- persistent activations ----
            q_t = bigsb.tile([128, S], f32r)
            k_t = bigsb.tile([128, skc], f32r)
            vaug = bigsb.tile([128, NT * 130], bf16)

            for _rep in range(reps):

                def emit_kv(c, lo=0, hi=None):
                    """K_T projection + V_T projection + V transpose + V_aug
                    assembly for key-chunk c, columns [lo, hi) of the chunk."""
                    cw = min(512, skc - c * 512) if hi is None else hi
                    ks = slice(c * 512 + lo, c * 512 + cw)
                    cw = cw - lo
                    psk = ps_misc.tile([128, 512], f32, tag="misc", name="psk")
                    for kt in range(4):
                        nc.tensor.matmul(
                            psk[:, :cw],
                            wk_sb[:, kt, :],
                            x2all[:, kt, ks],
                            start=(kt == 0),
                            stop=(kt == 3),
                        )
                    nc.vector.tensor_copy(k_t[:, ks], psk[:, :cw])
                    psvt = ps_misc.tile([128, 512], f32, tag="misc", name="psvt")
                    for kt in range(4):
                        nc.tensor.matmul(
                            psvt[:, :cw],
                            wv_sb[:, kt, :],
                            x2all[:, kt, ks],
                            start=(kt == 0),
                            stop=(kt == 3),
                        )
                    vt_sb = work.tile([128, 512], f32, tag="vt")
                    nc.vector.tensor_copy(vt_sb[:, :cw], psvt[:, :cw])
                    for j in range(cw // 128):
                        t = c * 4 + lo // 128 + j
                        psv = ps_misc.tile([128, 128], f32, tag="misc", name="psv")
                        nc.tensor.transpose(
                            psv, vt_sb[:, j * 128 : (j + 1) * 128], ident
                        )
                        o = t * 130
                        # pad keys are zero columns of x2c, so V pad rows are
                        # already zero; only the mask column (denominator
                        # guard) needs explicit values
                        nc.vector.tensor_copy(vaug[:, o : o + 64], psv[:, 0:64])
                        nc.vector.tensor_copy(
                            vaug[:, o + 64 : o + 65], maskb_sb[:, t : t + 1]
                        )
                        nc.vector.tensor_copy(
                            vaug[:, o + 65 : o + 129], psv[:, 64:128]
                        )
                        nc.vector.tensor_copy(
                            vaug[:, o + 129 : o + 130], maskb_sb[:, t : t + 1]
                        )

                def emit_qproj(c, x1c=None):
                    if x1c is None:
                        x1c = xstream.tile([128, 4, 512], bf16, tag="xs", name="x1c")
                        nc.sync.dma_start(
                            out=x1c, in_=x1r[:, :, c * 512 : (c + 1) * 512]
                        )
                    psq = ps_misc.tile([128, 512], f32, tag="misc", name="psq")
                    for kt in range(4):
                        nc.tensor.matmul(
                            psq,
                            wq_sb[:, kt, :],
                            x1c[:, kt, :],
                            start=(kt == 0),
                            stop=(kt == 3),
                        )
                    nc.vector.tensor_copy(q_t[:, c * 512 : (c + 1) * 512], psq)

                emit_qproj(0, x1c=x1c0 if _rep == 0 else None)
                # K projection for just the first key tile (128 cols) so the
                # first score matmul fires as soon as possible
                ksplit = min(128, skc)
                psk0 = ps_misc.tile([128, 128], f32, tag="misc", name="psk0")
                for kt in range(4):
                    nc.tensor.matmul(
                        psk0[:, :ksplit],
                        wk_sb[:, kt, :],
                        x2all[:, kt, 0:ksplit],
                        start=(kt == 0),
                        stop=(kt == 3),
                    )
                nc.vector.tensor_copy(k_t[:, 0:ksplit], psk0[:, :ksplit])

                def emit_scores_exp(c, t):
                    qs_c = slice(c * 512, (c + 1) * 512)
                    sc = ps_big.tile([128, 1024], f32, tag="sc", name="sc")
                    nc.tensor.matmul(
                        sc[:, 0:512],
                        r(k_t[0:64, t * 128 : (t + 1) * 128]),
                        r(q_t[0:64, qs_c]),
                        start=True,
                        stop=True,
                    )
                    nc.tensor.matmul(
                        sc[:, 512:1024],
                        r(k_t[64:128, t * 128 : (t + 1) * 128]),
                        r(q_t[64:128, qs_c]),
                        start=True,
                        stop=True,
                    )
                    pt = pexp.tile([128, 1024], bf16, name="pt")
                    nc.scalar.activation(out=pt, in_=sc, func=EXP, scale=0.125)
                    return pt

                def emit_av_group(j, h, pts, state):
                    """AV form B for query block j, head h of a chunk:
                    acc[128 q, 65] accumulated over all key tiles; col 64 is
                    the softmax denominator. Normalize, transpose, and stack
                    into the [128, 128] o_t tile for the output projection."""
                    if h == 0:
                        state["ot"] = work.tile([128, 128], f32r, tag="ot", bufs=3, name="ot")
                    acc = ps_acc.tile([128, 65], f32, tag="acc", name="acc")
                    qs = slice(h * 512 + j * 128, h * 512 + (j + 1) * 128)
                    for t in range(NT):
                        nc.tensor.matmul(
                            acc,
                            pts[t][:, qs],
                            vaug[:, t * 130 + h * 65 : t * 130 + h * 65 + 65],
                            start=(t == 0),
                            stop=(t == NT - 1),
                        )
                    recip = work.tile([128, 1], f32, tag="recip", bufs=3, name="recip")
                    nc.vector.reciprocal(recip, acc[:, 64:65])
                    o_sb = work.tile([128, 64], f32, tag="osb", bufs=3, name="o_sb")
                    nc.vector.tensor_scalar_mul(o_sb, acc[:, 0:64], recip)
                    ps_t = ps_misc.tile([64, 128], f32, tag="misc", name="ps_t")
                    nc.tensor.transpose(ps_t, o_sb, ident)
                    nc.vector.tensor_copy(
                        state["ot"][h * 64 : (h + 1) * 64, :], ps_t
                    )

                def emit_outproj(c, j, state):
                    tp = ps_misc.tile([128, 512], f32, tag="misc", name="tp")
                    nc.tensor.matmul(
                        tp, r(state["ot"]), r(wo2_sb), start=True, stop=True
                    )
                    out_sb = work.tile([128, 512], bf16, tag="outsb", bufs=4, name="out_sb")
                    nc.vector.tensor_copy(out_sb, tp)
                    st = c * 4 + j
                    nc.sync.dma_start(
                        out=out[st * 128 : (st + 1) * 128, :], in_=out_sb
                    )

                avq = []  # deferred AV/outproj work items for the prev chunk

                def enqueue_chunk_av(c, pts):
                    for j in range(4):
                        state = {}
                        for h in range(2):
                            avq.append(
                                lambda j=j, h=h, pts=pts, state=state: emit_av_group(
                                    j, h, pts, state
                                )
                            )
                        avq.append(
                            lambda c=c, j=j, state=state: emit_outproj(c, j, state)
                        )

                pt_carry = None  # exp output for (c, t=0) computed in chunk c-1
                for c in range(NQC):
                    pts = []
                    for t in range(NT):
                        if t == 0 and pt_carry is not None:
                            pt = pt_carry
                            pt_carry = None
                        else:
                            pt = emit_scores_exp(c, t)
                        pts.append(pt)
                        # stream later key projections into chunk 0, one
                        # 128-col piece per score slot (keeps ScalarE fed)
                        if c == 0 and t == 0 and skc > ksplit:
                            emit_kv(0, lo=0, hi=min(512, skc))
                        if c == 0 and t >= 1:
                            i = t + 3  # pieces 4..NT-1 at slots 1..NT-4
                            if i < NT:
                                emit_kv(i // 4, lo=(i % 4) * 128, hi=(i % 4) * 128 + 128)
                        if t == NT // 2 and c + 1 < NQC:
                            emit_qproj(c + 1)
                        if t == NT - 1 and c + 1 < NQC:
                            pt_carry = emit_scores_exp(c + 1, 0)
                        # drain one deferred AV/outproj item per score slot
                        if avq:
                            avq.pop(0)()
                    # any leftovers (short NT) before enqueueing the new chunk
                    while avq:
                        avq.pop(0)()
                    enqueue_chunk_av(c, pts)
                while avq:
                    avq.pop(0)()

    nc.compile()
    return nc


def _get_runtime(skc: int, reps: int = 1):
    key = (skc, reps)
    if key not in _RUNTIMES:
        _RUNTIMES[key] = _build_program(skc, reps)
    return _RUNTIMES[key]


def _numpy_reference(x1, x2, mask, Wq, bq, Wk, bk, Wv, bv, Wo, bo):
    q = (x1 @ Wq + bq).reshape(B, S, H, DH).transpose(0, 2, 1, 3)
    k = (x2 @ Wk + bk).reshape(B, S, H, DH).transpose(0, 2, 1, 3)
    v = (x2 @ Wv + bv).reshape(B, S, H, DH).transpose(0, 2, 1, 3)
    scores = np.einsum("bhqd,bhkd->bhqk", q, k) / np.sqrt(np.float32(DH))
    scores = scores + mask[:, None, None, :].astype(np.float32) * np.float32(-1e9)
    scores = scores - scores.max(axis=-1, keepdims=True)
    e = np.exp(scores)
    attn = e / e.sum(axis=-1, keepdims=True)
    o = np.einsum("bhqk,bhkd->bhqd", attn, v)
    o = o.transpose(0, 2, 1, 3).reshape(B, S, D)
    return (o @ Wo + bo).astype(np.float32)


def _make_in_maps(x1, x2, mask, Wq, Wk, Wv, Wo):
    import ml_dtypes

    bf16 = ml_dtypes.bfloat16
    keep = [np.nonzero(mask[b] == 0)[0] for b in range(B)]
    counts = [len(k) for k in keep]
    skc = ((max(counts) + 127) // 128) * 128
    nt = skc // 128
    in_maps = []
    for c in range(NCORES):
        b, hp = c // 4, c % 4
        x2c = np.zeros((skc, D), dtype=np.float32)
        x2c[: counts[b]] = x2[b][keep[b]]
        mf = np.zeros((nt, 128), dtype=np.float32)
        mf.reshape(-1)[: counts[b]] = 1.0
        cols = slice(hp * 128, (hp + 1) * 128)
        in_maps.append(
            {
                "x1t": np.ascontiguousarray(x1[b].T).astype(bf16),
                "x2ct": np.ascontiguousarray(x2c.T).astype(bf16),
                "maskb": np.ascontiguousarray(mf.T).astype(bf16),
                "wq": np.ascontiguousarray(Wq[:, cols]).astype(bf16),
                "wk": np.ascontiguousarray(Wk[:, cols]).astype(bf16),
                "wv": np.ascontiguousarray(Wv[:, cols]).astype(bf16),
                "wo2": np.ascontiguousarray(Wo[hp * 128 : (hp + 1) * 128, :]),
            }
        )
    return skc, in_maps


def kernel(x1, x2, mask, Wq, bq, Wk, bk, Wv, bv, Wo, bo):
    from concourse.bass_utils import run_bass_kernel_spmd

    x1 = np.asarray(x1, dtype=np.float32)
    x2 = np.asarray(x2, dtype=np.float32)
    mask = np.asarray(mask)
    Wq = np.asarray(Wq, dtype=np.float32)
    Wk = np.asarray(Wk, dtype=np.float32)
    Wv = np.asarray(Wv, dtype=np.float32)
    Wo = np.asarray(Wo, dtype=np.float32)
    bq, bk, bv, bo = (np.asarray(b, dtype=np.float32) for b in (bq, bk, bv, bo))

    counts = [int((mask[b] == 0).sum()) for b in range(B)]
    if any(np.abs(b).max() > 0 for b in (bq, bk, bv) if b.size) or min(counts) == 0:
        return _numpy_reference(x1, x2, mask, Wq, bq, Wk, bk, Wv, bv, Wo, bo)

    skc, in_maps = _make_in_maps(x1, x2, mask, Wq, Wk, Wv, Wo)
    nc = _get_runtime(skc)

    res = run_bass_kernel_spmd(nc, in_maps, core_ids=list(range(NCORES)))
    full = np.empty((B, S, D), dtype=np.float32)
    for b in range(B):
        acc = res.results[4 * b]["out"].astype(np.float32)
        for hp in range(1, 4):
            acc = acc + res.results[4 * b + hp]["out"].astype(np.float32)
        full[b] = acc + bo
    return full


# revision 8
# speedup vs baseline: 1.0278x; 1.0278x over previous
"""Trainium2 Bass kernel for MultiHeadAttention (B=2, S=4096, D=512, H=8).

Sharding: 16 (batch, head) units across 8 cores -> each core owns one batch
and a contiguous pair of heads (2 heads x 64 depth = 128 columns of the
QKV projections, 128 rows of the output projection).

Key ideas:
  * Mask compression on host: keys with mask==1 receive -1e9 before softmax,
    so their probability is exactly 0 in fp32. We drop those keys entirely
    (gather unmasked rows of x2), roughly halving scores/softmax/AV work.
    Dropped-key handling is exact, not approximate.
  * Scores run out of a transposed activation layout:
      Q_T, K_T: [128(=2 heads x 64 depth), S]  (from x1^T / x2c^T inputs)
    scores for one key-tile land as [128 keys, 1024(=2 heads x 512 queries)]
    in PSUM, and a single ScalarE activation does exp(scores/8) PSUM->SBUF
    into bf16 P tiles.
  * AV uses the "form B" orientation: out[128 queries, 65] accumulated over
    key tiles with the P tile as the (bf16) stationary operand and the
    65-column V^T-with-mask-column tile as the moving operand. Cost-wise the
    moving free size is 65 instead of 512, halving the PE time of AV
    relative to the V^T @ P orientation. Column 64 accumulates the softmax
    denominator (mask column rides along in V).
  * Normalization is a per-query scalar: reciprocal of column 64 and one
    VectorE scalar-mul -> bf16 O block [128 q, 64]. A PE transpose brings it
    to [64, 128 q]; both heads stack into a [128, 128] tile so the output
    projection is a single 128-contraction matmul per 128-row output block
    (both heads reduced in one pass).
  * All fp32 matmul operands are float32r (the PE's fast single-pass fp32
    mode); P/V/O/Wo run in bf16 (the loose 2e-2 tolerance gives >10x
    headroom for bf16's ~0.1% rounding).
  * Output is written in bf16 (halves the store DMA); host sums the 4
    per-core partials of each batch in fp32 and adds bo.

Non-zero q/k/v biases or an all-masked batch fall back to a numpy reference
(those inputs cannot occur with the problem's setup_inputs).
"""

import numpy as np

B, S, D, H = 2, 4096, 512, 8
DH = 64  # depth per head
NCORES = 8

_RUNTIMES = {}


def _build_program(skc: int, reps: int = 1):
    """Build the per-core Bass program. skc = padded compressed key count."""
    import concourse.bacc as bacc
    import concourse.mybir as mybir
    from concourse.masks import make_identity
    from concourse.tile import TileContext

    f32 = mybir.dt.float32
    f32r = mybir.dt.float32r
    bf16 = mybir.dt.bfloat16
    EXP = mybir.ActivationFunctionType.Exp
    r = lambda ap: ap.bitcast(mybir.dt.float32r)  # fast fp32 matmul mode

    NT = skc // 128  # key tiles
    NQC = S // 512  # query chunks (512 wide)
    NKC = (skc + 511) // 512  # key chunks for the K/V projections

    nc = bacc.Bacc("TRN2", target_bir_lowering=False, debug=False, num_devices=NCORES)

    x1t = nc.dram_tensor("x1t", [D, S], bf16, kind="ExternalInput")
    x2ct = nc.dram_tensor("x2ct", [D, skc], bf16, kind="ExternalInput")
    maskb = nc.dram_tensor("maskb", [128, NT], bf16, kind="ExternalInput")
    wq = nc.dram_tensor("wq", [D, 128], bf16, kind="ExternalInput")
    wk = nc.dram_tensor("wk", [D, 128], bf16, kind="ExternalInput")
    wv = nc.dram_tensor("wv", [D, 128], bf16, kind="ExternalInput")
    wo2 = nc.dram_tensor("wo2", [128, 512], f32r, kind="ExternalInput")
    out = nc.dram_tensor("out", [S, D], bf16, kind="ExternalOutput")

    with nc.allow_low_precision(
        reason="bf16 P/V/O tiles; fp32 PSUM accumulation; 2e-2 tolerance"
    ), TileContext(nc) as tc:
        with (
            tc.tile_pool(name="consts", bufs=1) as consts,
            tc.tile_pool(name="bigsb", bufs=1) as bigsb,
            tc.tile_pool(name="xstream", bufs=3) as xstream,
            tc.tile_pool(name="pexp", bufs=34) as pexp,
            tc.tile_pool(name="work", bufs=3) as work,
            tc.tile_pool(name="ps_big", bufs=2, space="PSUM") as ps_big,
            tc.tile_pool(name="ps_acc", bufs=2, space="PSUM") as ps_acc,
            tc.tile_pool(name="ps_misc", bufs=2, space="PSUM") as ps_misc,
        ):
            # ---- constants / persistent buffers (DMA issue order matters:
            # the DMA device drains them in order) ----
            # x1 chunk 0 first, split per k-tile so the first Q matmul can
            # start after only a quarter of the transfer
            x1r = x1t.rearrange("(t p) s -> p t s", p=128)
            wq_sb = consts.tile([128, 4, 128], bf16)
            nc.sync.dma_start(out=wq_sb, in_=wq.rearrange("(t p) m -> p t m", p=128))
            wk_sb = consts.tile([128, 4, 128], bf16)
            nc.sync.dma_start(out=wk_sb, in_=wk.rearrange("(t p) m -> p t m", p=128))
            x1c0 = xstream.tile([128, 4, 512], bf16, tag="xs")
            for kt in range(4):
                nc.sync.dma_start(out=x1c0[:, kt, :], in_=x1r[:, kt, 0:512])
            x2all = bigsb.tile([128, 4, skc], bf16)
            x2r = x2ct.rearrange("(t p) s -> p t s", p=128)
            c0w = min(512, skc)
            c0a = min(128, c0w)  # first key-tile lands fast -> early first score
            nc.sync.dma_start(out=x2all[:, :, 0:c0a], in_=x2r[:, :, 0:c0a])
            wv_sb = consts.tile([128, 4, 128], bf16)
            nc.sync.dma_start(out=wv_sb, in_=wv.rearrange("(t p) m -> p t m", p=128))
            maskb_sb = consts.tile([128, NT], bf16)
            nc.sync.dma_start(out=maskb_sb, in_=maskb[:, :])
            if c0w > c0a:
                nc.sync.dma_start(out=x2all[:, :, c0a:c0w], in_=x2r[:, :, c0a:c0w])
            for c in range(1, NKC):
                cw = min(512, skc - c * 512)
                nc.sync.dma_start(
                    out=x2all[:, :, c * 512 : c * 512 + cw],
                    in_=x2r[:, :, c * 512 : c * 512 + cw],
                )
            wo2_sb = consts.tile([128, 512], f32r)
            nc.sync.dma_start(out=wo2_sb, in_=wo2[:, :])

            ident = consts.tile([128, 128], f32)
            make_identity(nc, ident)

            # ---- persistent activations ----
            q_t = bigsb.tile([128, S], f32r)
            k_t = bigsb.tile([128, skc], f32r)
            vaug = bigsb.tile([128, NT * 130], bf16)

            for _rep in range(reps):

                def kv_kproj(c, lo, cw):
                    ks = slice(c * 512 + lo, c * 512 + lo + cw)
                    psk = ps_misc.tile([128, 512], f32, tag="misc", name="psk")
                    for kt in range(4):
                        nc.tensor.matmul(
                            psk[:, :cw],
                            wk_sb[:, kt, :],
                            x2all[:, kt, ks],
                            start=(kt == 0),
                            stop=(kt == 3),
                        )
                    nc.vector.tensor_copy(k_t[:, ks], psk[:, :cw])

                def kv_vproj(c, lo, cw, state):
                    ks = slice(c * 512 + lo, c * 512 + lo + cw)
                    psvt = ps_misc.tile([128, 512], f32, tag="misc", name="psvt")
                    for kt in range(4):
                        nc.tensor.matmul(
                            psvt[:, :cw],
                            wv_sb[:, kt, :],
                            x2all[:, kt, ks],
                            start=(kt == 0),
                            stop=(kt == 3),
                        )
                    vt_sb = work.tile([128, 512], f32, tag="vt", name="vt_sb")
                    nc.vector.tensor_copy(vt_sb[:, :cw], psvt[:, :cw])
                    state["vt"] = vt_sb

                def kv_vaug(c, lo, cw, state, j0, j1):
                    vt_sb = state["vt"]
                    for j in range(j0, min(j1, cw // 128)):
                        t = c * 4 + lo // 128 + j
                        psv = ps_misc.tile([128, 128], f32, tag="misc", name="psv")
                        nc.tensor.transpose(
                            psv, vt_sb[:, j * 128 : (j + 1) * 128], ident
                        )
                        o = t * 130
                        # pad keys are zero columns of x2c, so V pad rows are
                        # already zero; only the mask column (denominator
                        # guard) needs explicit values
                        nc.vector.tensor_copy(vaug[:, o : o + 64], psv[:, 0:64])
                        nc.vector.tensor_copy(
                            vaug[:, o + 64 : o + 65], maskb_sb[:, t : t + 1]
                        )
                        nc.vector.tensor_copy(
                            vaug[:, o + 65 : o + 129], psv[:, 64:128]
                        )
                        nc.vector.tensor_copy(
                            vaug[:, o + 129 : o + 130], maskb_sb[:, t : t + 1]
                        )

                def emit_kv(c, lo=0, hi=None):
                    cw = (min(512, skc - c * 512) if hi is None else hi) - lo
                    state = {}
                    kv_kproj(c, lo, cw)
                    kv_vproj(c, lo, cw, state)
                    kv_vaug(c, lo, cw, state, 0, 4)

                def emit_qproj(c, x1c=None):
                    if x1c is None:
                        x1c = xstream.tile([128, 4, 512], bf16, tag="xs", name="x1c")
                        nc.sync.dma_start(
                            out=x1c, in_=x1r[:, :, c * 512 : (c + 1) * 512]
                        )
                    psq = ps_misc.tile([128, 512], f32, tag="misc", name="psq")
                    for kt in range(4):
                        nc.tensor.matmul(
                            psq,
                            wq_sb[:, kt, :],
                            x1c[:, kt, :],
                            start=(kt == 0),
                            stop=(kt == 3),
                        )
                    nc.vector.tensor_copy(q_t[:, c * 512 : (c + 1) * 512], psq)

                emit_qproj(0, x1c=x1c0 if _rep == 0 else None)
                # K projection for just the first key tile (128 cols) so the
                # first score matmul fires as soon as possible
                ksplit = min(128, skc)
                psk0 = ps_misc.tile([128, 128], f32, tag="misc", name="psk0")
                for kt in range(4):
                    nc.tensor.matmul(
                        psk0[:, :ksplit],
                        wk_sb[:, kt, :],
                        x2all[:, kt, 0:ksplit],
                        start=(kt == 0),
                        stop=(kt == 3),
                    )
                nc.vector.tensor_copy(k_t[:, 0:ksplit], psk0[:, :ksplit])

                def emit_scores_exp(c, t):
                    qs_c = slice(c * 512, (c + 1) * 512)
                    sc = ps_big.tile([128, 1024], f32, tag="sc", name="sc")
                    nc.tensor.matmul(
                        sc[:, 0:512],
                        r(k_t[0:64, t * 128 : (t + 1) * 128]),
                        r(q_t[0:64, qs_c]),
                        start=True,
                        stop=True,
                    )
                    nc.tensor.matmul(
                        sc[:, 512:1024],
                        r(k_t[64:128, t * 128 : (t + 1) * 128]),
                        r(q_t[64:128, qs_c]),
                        start=True,
                        stop=True,
                    )
                    pt = pexp.tile([128, 1024], bf16, name="pt")
                    nc.scalar.activation(out=pt, in_=sc, func=EXP, scale=0.125)
                    return pt

                def emit_av_group(j, h, pts, state):
                    """AV form B for query block j, head h of a chunk:
                    acc[128 q, 65] accumulated over all key tiles; col 64 is
                    the softmax denominator. Normalize, transpose, and stack
                    into the [128, 128] o_t tile for the output projection."""
                    if h == 0:
                        state["ot"] = work.tile([128, 128], f32r, tag="ot", bufs=3, name="ot")
                    acc = ps_acc.tile([128, 65], f32, tag="acc", name="acc")
                    qs = slice(h * 512 + j * 128, h * 512 + (j + 1) * 128)
                    for t in range(NT):
                        nc.tensor.matmul(
                            acc,
                            pts[t][:, qs],
                            vaug[:, t * 130 + h * 65 : t * 130 + h * 65 + 65],
                            start=(t == 0),
                            stop=(t == NT - 1),
                        )
                    recip = work.tile([128, 1], f32, tag="recip", bufs=3, name="recip")
                    nc.vector.reciprocal(recip, acc[:, 64:65])
                    o_sb = work.tile([128, 64], f32, tag="osb", bufs=3, name="o_sb")
                    nc.vector.tensor_scalar_mul(o_sb, acc[:, 0:64], recip)
                    ps_t = ps_misc.tile([64, 128], f32, tag="misc", name="ps_t")
                    nc.tensor.transpose(ps_t, o_sb, ident)
                    nc.vector.tensor_copy(
                        state["ot"][h * 64 : (h + 1) * 64, :], ps_t
                    )

                def emit_outproj(c, j, state):
                    tp = ps_misc.tile([128, 512], f32, tag="misc", name="tp")
                    nc.tensor.matmul(
                        tp, r(state["ot"]), r(wo2_sb), start=True, stop=True
                    )
                    out_sb = work.tile([128, 512], bf16, tag="outsb", bufs=4, name="out_sb")
                    nc.vector.tensor_copy(out_sb, tp)
                    st = c * 4 + j
                    nc.sync.dma_start(
                        out=out[st * 128 : (st + 1) * 128, :], in_=out_sb
                    )

                avq = []  # deferred AV/outproj work items for the prev chunk

                def enqueue_chunk_av(c, pts):
                    for j in range(4):
                        state = {}
                        for h in range(2):
                            avq.append(
                                lambda j=j, h=h, pts=pts, state=state: emit_av_group(
                                    j, h, pts, state
                                )
                            )
                        avq.append(
                            lambda c=c, j=j, state=state: emit_outproj(c, j, state)
                        )

                pt_carry = None  # exp output for (c, t=0) computed in chunk c-1
                kv_states = {}
                for c in range(NQC):
                    pts = []
                    for t in range(NT):
                        if t == 0 and pt_carry is not None:
                            pt = pt_carry
                            pt_carry = None
                        else:
                            pt = emit_scores_exp(c, t)
                        pts.append(pt)
                        # stream later key projections into chunk 0; each
                        # 512-col kv chunk is split into 4 sub-emissions
                        # spread over consecutive score slots
                        if c == 0 and t == 0 and skc > ksplit:
                            emit_kv(0, lo=0, hi=min(512, skc))
                        if c == 0 and t >= 1 and (kc := (t - 1) // 4 + 1) < NKC:
                            ph = (t - 1) % 4
                            cw = min(512, skc - kc * 512)
                            st = kv_states.setdefault(kc, {})
                            if ph == 0:
                                kv_kproj(kc, 0, cw)
                            elif ph == 1:
                                kv_vproj(kc, 0, cw, st)
                            elif ph == 2:
                                kv_vaug(kc, 0, cw, st, 0, 2)
                            else:
                                kv_vaug(kc, 0, cw, st, 2, 4)
                        if t == NT // 2 and c + 1 < NQC:
                            emit_qproj(c + 1)
                        if t == NT - 1 and c + 1 < NQC:
                            pt_carry = emit_scores_exp(c + 1, 0)
                        # drain one deferred AV/outproj item per score slot
                        if avq:
                            avq.pop(0)()
                    # any leftovers (short NT) before enqueueing the new chunk
                    while avq:
                        avq.pop(0)()
                    enqueue_chunk_av(c, pts)
                while avq:
                    avq.pop(0)()

    nc.compile()
    return nc


def _get_runtime(skc: int, reps: int = 1):
    key = (skc, reps)
    if key not in _RUNTIMES:
        _RUNTIMES[key] = _build_program(skc, reps)
    return _RUNTIMES[key]


def _numpy_reference(x1, x2, mask, Wq, bq, Wk, bk, Wv, bv, Wo, bo):
    q = (x1 @ Wq + bq).reshape(B, S, H, DH).transpose(0, 2, 1, 3)
    k = (x2 @ Wk + bk).reshape(B, S, H, DH).transpose(0, 2, 1, 3)
    v = (x2 @ Wv + bv).reshape(B, S, H, DH).transpose(0, 2, 1, 3)
    scores = np.einsum("bhqd,bhkd->bhqk", q, k) / np.sqrt(np.float32(DH))
    scores = scores + mask[:, None, None, :].astype(np.float32) * np.float32(-1e9)
    scores = scores - scores.max(axis=-1, keepdims=True)
    e = np.exp(scores)
    attn = e / e.sum(axis=-1, keepdims=True)
    o = np.einsum("bhqk,bhkd->bhqd", attn, v)
    o = o.transpose(0, 2, 1, 3).reshape(B, S, D)
    return (o @ Wo + bo).astype(np.float32)


def _make_in_maps(x1, x2, mask, Wq, Wk, Wv, Wo):
    import ml_dtypes

    bf16 = ml_dtypes.bfloat16
    keep = [np.nonzero(mask[b] == 0)[0] for b in range(B)]
    counts = [len(k) for k in keep]
    skc = ((max(counts) + 127) // 128) * 128
    nt = skc // 128
    in_maps = []
    for c in range(NCORES):
        b, hp = c // 4, c % 4
        x2c = np.zeros((skc, D), dtype=np.float32)
        x2c[: counts[b]] = x2[b][keep[b]]
        mf = np.zeros((nt, 128), dtype=np.float32)
        mf.reshape(-1)[: counts[b]] = 1.0
        cols = slice(hp * 128, (hp + 1) * 128)
        in_maps.append(
            {
                "x1t": np.ascontiguousarray(x1[b].T).astype(bf16),
                "x2ct": np.ascontiguousarray(x2c.T).astype(bf16),
                "maskb": np.ascontiguousarray(mf.T).astype(bf16),
                "wq": np.ascontiguousarray(Wq[:, cols]).astype(bf16),
                "wk": np.ascontiguousarray(Wk[:, cols]).astype(bf16),
                "wv": np.ascontiguousarray(Wv[:, cols]).astype(bf16),
                "wo2": np.ascontiguousarray(Wo[hp * 128 : (hp + 1) * 128, :]),
            }
        )
    return skc, in_maps


def kernel(x1, x2, mask, Wq, bq, Wk, bk, Wv, bv, Wo, bo):
    from concourse.bass_utils import run_bass_kernel_spmd

    x1 = np.asarray(x1, dtype=np.float32)
    x2 = np.asarray(x2, dtype=np.float32)
    mask = np.asarray(mask)
    Wq = np.asarray(Wq, dtype=np.float32)
    Wk = np.asarray(Wk, dtype=np.float32)
    Wv = np.asarray(Wv, dtype=np.float32)
    Wo = np.asarray(Wo, dtype=np.float32)
    bq, bk, bv, bo = (np.asarray(b, dtype=np.float32) for b in (bq, bk, bv, bo))

    counts = [int((mask[b] == 0).sum()) for b in range(B)]
    if any(np.abs(b).max() > 0 for b in (bq, bk, bv) if b.size) or min(counts) == 0:
        return _numpy_reference(x1, x2, mask, Wq, bq, Wk, bk, Wv, bv, Wo, bo)

    skc, in_maps = _make_in_maps(x1, x2, mask, Wq, Wk, Wv, Wo)
    nc = _get_runtime(skc)

    res = run_bass_kernel_spmd(nc, in_maps, core_ids=list(range(NCORES)))
    full = np.empty((B, S, D), dtype=np.float32)
    for b in range(B):
        acc = res.results[4 * b]["out"].astype(np.float32)
        for hp in range(1, 4):
            acc = acc + res.results[4 * b + hp]["out"].astype(np.float32)
        full[b] = acc + bo
    return full


# revision 9
# speedup vs baseline: 1.0390x; 1.0108x over previous
"""Trainium2 Bass kernel for MultiHeadAttention (B=2, S=4096, D=512, H=8).

Sharding: 16 (batch, head) units across 8 cores -> each core owns one batch
and a contiguous pair of heads (2 heads x 64 depth = 128 columns of the
QKV projections, 128 rows of the output projection).

Key ideas:
  * Mask compression on host: keys with mask==1 receive -1e9 before softmax,
    so their probability is exactly 0 in fp32. We drop those keys entirely
    (gather unmasked rows of x2), roughly halving scores/softmax/AV work.
    Dropped-key handling is exact, not approximate.
  * Scores run out of a transposed activation layout:
      Q_T, K_T: [128(=2 heads x 64 depth), S]  (from x1^T / x2c^T inputs)
    scores for one key-tile land as [128 keys, 1024(=2 heads x 512 queries)]
    in PSUM, and a single ScalarE activation does exp(scores/8) PSUM->SBUF
    into bf16 P tiles.
  * AV uses the "form B" orientation: out[128 queries, 65] accumulated over
    key tiles with the P tile as the (bf16) stationary operand and the
    65-column V^T-with-mask-column tile as the moving operand. Cost-wise the
    moving free size is 65 instead of 512, halving the PE time of AV
    relative to the V^T @ P orientation. Column 64 accumulates the softmax
    denominator (mask column rides along in V).
  * Normalization is a per-query scalar: reciprocal of column 64 and one
    VectorE scalar-mul -> bf16 O block [128 q, 64]. A PE transpose brings it
    to [64, 128 q]; both heads stack into a [128, 128] tile so the output
    projection is a single 128-contraction matmul per 128-row output block
    (both heads reduced in one pass).
  * All fp32 matmul operands are float32r (the PE's fast single-pass fp32
    mode); P/V/O/Wo run in bf16 (the loose 2e-2 tolerance gives >10x
    headroom for bf16's ~0.1% rounding).
  * Output is written in bf16 (halves the store DMA); host sums the 4
    per-core partials of each batch in fp32 and adds bo.

Non-zero q/k/v biases or an all-masked batch fall back to a numpy reference
(those inputs cannot occur with the problem's setup_inputs).
"""

import numpy as np

B, S, D, H = 2, 4096, 512, 8
DH = 64  # depth per head
NCORES = 8

_RUNTIMES = {}


def _build_program(skc: int, reps: int = 1):
    """Build the per-core Bass program. skc = padded compressed key count."""
    import concourse.bacc as bacc
    import concourse.mybir as mybir
    from concourse.masks import make_identity
    from concourse.tile import TileContext

    f32 = mybir.dt.float32
    f32r = mybir.dt.float32r
    bf16 = mybir.dt.bfloat16
    EXP = mybir.ActivationFunctionType.Exp
    r = lambda ap: ap.bitcast(mybir.dt.float32r)  # fast fp32 matmul mode

    NT = skc // 128  # key tiles
    NQC = S // 512  # query chunks (512 wide)
    NKC = (skc + 511) // 512  # key chunks for the K/V projections

    nc = bacc.Bacc("TRN2", target_bir_lowering=False, debug=False, num_devices=NCORES)

    x1t = nc.dram_tensor("x1t", [D, S], bf16, kind="ExternalInput")
    x2ct = nc.dram_tensor("x2ct", [D, skc], bf16, kind="ExternalInput")
    maskb = nc.dram_tensor("maskb", [128, NT], bf16, kind="ExternalInput")
    wq = nc.dram_tensor("wq", [D, 128], bf16, kind="ExternalInput")
    wk = nc.dram_tensor("wk", [D, 128], bf16, kind="ExternalInput")
    wv = nc.dram_tensor("wv", [D, 128], bf16, kind="ExternalInput")
    wo2 = nc.dram_tensor("wo2", [128, 512], f32r, kind="ExternalInput")
    out = nc.dram_tensor("out", [S, D], bf16, kind="ExternalOutput")

    with nc.allow_low_precision(
        reason="bf16 P/V/O tiles; fp32 PSUM accumulation; 2e-2 tolerance"
    ), TileContext(nc) as tc:
        with (
            tc.tile_pool(name="consts", bufs=1) as consts,
            tc.tile_pool(name="bigsb", bufs=1) as bigsb,
            tc.tile_pool(name="xstream", bufs=3) as xstream,
            tc.tile_pool(name="pexp", bufs=34) as pexp,
            tc.tile_pool(name="work", bufs=3) as work,
            tc.tile_pool(name="ps_big", bufs=2, space="PSUM") as ps_big,
            tc.tile_pool(name="ps_acc", bufs=2, space="PSUM") as ps_acc,
            tc.tile_pool(name="ps_misc", bufs=2, space="PSUM") as ps_misc,
        ):
            # ---- constants / persistent buffers (DMA issue order matters:
            # the DMA device drains them in order) ----
            # x1 chunk 0 first, split per k-tile so the first Q matmul can
            # start after only a quarter of the transfer
            x1r = x1t.rearrange("(t p) s -> p t s", p=128)
            wq_sb = consts.tile([128, 4, 128], bf16)
            nc.sync.dma_start(out=wq_sb, in_=wq.rearrange("(t p) m -> p t m", p=128))
            wk_sb = consts.tile([128, 4, 128], bf16)
            nc.sync.dma_start(out=wk_sb, in_=wk.rearrange("(t p) m -> p t m", p=128))
            x1c0 = xstream.tile([128, 4, 512], bf16, tag="xs")
            for kt in range(4):
                nc.sync.dma_start(out=x1c0[:, kt, :], in_=x1r[:, kt, 0:512])
            x2all = bigsb.tile([128, 4, skc], bf16)
            x2r = x2ct.rearrange("(t p) s -> p t s", p=128)
            c0w = min(512, skc)
            c0a = min(128, c0w)  # first key-tile lands fast -> early first score
            nc.sync.dma_start(out=x2all[:, :, 0:c0a], in_=x2r[:, :, 0:c0a])
            wv_sb = consts.tile([128, 4, 128], bf16)
            nc.sync.dma_start(out=wv_sb, in_=wv.rearrange("(t p) m -> p t m", p=128))
            maskb_sb = consts.tile([128, NT], bf16)
            nc.sync.dma_start(out=maskb_sb, in_=maskb[:, :])
            if c0w > c0a:
                nc.sync.dma_start(out=x2all[:, :, c0a:c0w], in_=x2r[:, :, c0a:c0w])
            for c in range(1, NKC):
                cw = min(512, skc - c * 512)
                nc.sync.dma_start(
                    out=x2all[:, :, c * 512 : c * 512 + cw],
                    in_=x2r[:, :, c * 512 : c * 512 + cw],
                )
            wo2_sb = consts.tile([128, 512], f32r)
            nc.sync.dma_start(out=wo2_sb, in_=wo2[:, :])

            ident = consts.tile([128, 128], f32)
            make_identity(nc, ident)

            # ---- persistent activations ----
            q_t = bigsb.tile([128, S], f32r)
            k_t = bigsb.tile([128, skc], f32r)
            vaug = bigsb.tile([128, NT * 130], bf16)

            for _rep in range(reps):

                def kv_kproj(c, lo, cw):
                    ks = slice(c * 512 + lo, c * 512 + lo + cw)
                    psk = ps_misc.tile([128, 512], f32, tag="misc", name="psk")
                    for kt in range(4):
                        nc.tensor.matmul(
                            psk[:, :cw],
                            wk_sb[:, kt, :],
                            x2all[:, kt, ks],
                            start=(kt == 0),
                            stop=(kt == 3),
                        )
                    nc.vector.tensor_copy(k_t[:, ks], psk[:, :cw])

                def kv_vproj(c, lo, cw, state):
                    ks = slice(c * 512 + lo, c * 512 + lo + cw)
                    psvt = ps_misc.tile([128, 512], f32, tag="misc", name="psvt")
                    for kt in range(4):
                        nc.tensor.matmul(
                            psvt[:, :cw],
                            wv_sb[:, kt, :],
                            x2all[:, kt, ks],
                            start=(kt == 0),
                            stop=(kt == 3),
                        )
                    vt_sb = work.tile([128, 512], f32, tag="vt", name="vt_sb")
                    nc.vector.tensor_copy(vt_sb[:, :cw], psvt[:, :cw])
                    state["vt"] = vt_sb

                def kv_vaug(c, lo, cw, state, j0, j1):
                    vt_sb = state["vt"]
                    for j in range(j0, min(j1, cw // 128)):
                        t = c * 4 + lo // 128 + j
                        psv = ps_misc.tile([128, 128], f32, tag="misc", name="psv")
                        nc.tensor.transpose(
                            psv, vt_sb[:, j * 128 : (j + 1) * 128], ident
                        )
                        o = t * 130
                        # pad keys are zero columns of x2c, so V pad rows are
                        # already zero; only the mask column (denominator
                        # guard) needs explicit values
                        nc.vector.tensor_copy(vaug[:, o : o + 64], psv[:, 0:64])
                        nc.vector.tensor_copy(
                            vaug[:, o + 64 : o + 65], maskb_sb[:, t : t + 1]
                        )
                        nc.vector.tensor_copy(
                            vaug[:, o + 65 : o + 129], psv[:, 64:128]
                        )
                        nc.vector.tensor_copy(
                            vaug[:, o + 129 : o + 130], maskb_sb[:, t : t + 1]
                        )

                def emit_kv(c, lo=0, hi=None):
                    cw = (min(512, skc - c * 512) if hi is None else hi) - lo
                    state = {}
                    kv_kproj(c, lo, cw)
                    kv_vproj(c, lo, cw, state)
                    kv_vaug(c, lo, cw, state, 0, 4)

                def emit_qproj(c, x1c=None):
                    if x1c is None:
                        x1c = xstream.tile([128, 4, 512], bf16, tag="xs", name="x1c")
                        nc.sync.dma_start(
                            out=x1c, in_=x1r[:, :, c * 512 : (c + 1) * 512]
                        )
                    psq = ps_misc.tile([128, 512], f32, tag="misc", name="psq")
                    for kt in range(4):
                        nc.tensor.matmul(
                            psq,
                            wq_sb[:, kt, :],
                            x1c[:, kt, :],
                            start=(kt == 0),
                            stop=(kt == 3),
                        )
                    nc.vector.tensor_copy(q_t[:, c * 512 : (c + 1) * 512], psq)

                emit_qproj(0, x1c=x1c0 if _rep == 0 else None)
                # K projection for just the first key tile (128 cols) so the
                # first score matmul fires as soon as possible
                ksplit = min(128, skc)
                psk0 = ps_misc.tile([128, 128], f32, tag="misc", name="psk0")
                for kt in range(4):
                    nc.tensor.matmul(
                        psk0[:, :ksplit],
                        wk_sb[:, kt, :],
                        x2all[:, kt, 0:ksplit],
                        start=(kt == 0),
                        stop=(kt == 3),
                    )
                nc.vector.tensor_copy(k_t[:, 0:ksplit], psk0[:, :ksplit])

                def emit_scores_exp(c, t):
                    qs_c = slice(c * 512, (c + 1) * 512)
                    sc = ps_big.tile([128, 1024], f32, tag="sc", name="sc")
                    nc.tensor.matmul(
                        sc[:, 0:512],
                        r(k_t[0:64, t * 128 : (t + 1) * 128]),
                        r(q_t[0:64, qs_c]),
                        start=True,
                        stop=True,
                    )
                    nc.tensor.matmul(
                        sc[:, 512:1024],
                        r(k_t[64:128, t * 128 : (t + 1) * 128]),
                        r(q_t[64:128, qs_c]),
                        start=True,
                        stop=True,
                    )
                    pt = pexp.tile([128, 1024], bf16, name="pt")
                    nc.scalar.activation(out=pt, in_=sc, func=EXP, scale=0.125)
                    return pt

                def emit_av_group(j, h, pts, state):
                    """AV form B for query block j, head h of a chunk:
                    acc[128 q, 65] accumulated over all key tiles; col 64 is
                    the softmax denominator. Normalize, transpose, and stack
                    into the [128, 128] o_t tile for the output projection."""
                    if h == 0:
                        state["ot"] = work.tile([128, 128], f32r, tag="ot", bufs=3, name="ot")
                    acc = ps_acc.tile([128, 65], f32, tag="acc", name="acc")
                    qs = slice(h * 512 + j * 128, h * 512 + (j + 1) * 128)
                    for t in range(NT):
                        nc.tensor.matmul(
                            acc,
                            pts[t][:, qs],
                            vaug[:, t * 130 + h * 65 : t * 130 + h * 65 + 65],
                            start=(t == 0),
                            stop=(t == NT - 1),
                        )
                    recip = work.tile([128, 1], f32, tag="recip", bufs=3, name="recip")
                    nc.vector.reciprocal(recip, acc[:, 64:65])
                    o_sb = work.tile([128, 64], f32, tag="osb", bufs=3, name="o_sb")
                    nc.vector.tensor_scalar_mul(o_sb, acc[:, 0:64], recip)
                    ps_t = ps_misc.tile([64, 128], f32, tag="misc", name="ps_t")
                    nc.tensor.transpose(ps_t, o_sb, ident)
                    nc.vector.tensor_copy(
                        state["ot"][h * 64 : (h + 1) * 64, :], ps_t
                    )

                def emit_outproj(c, j, state):
                    tp = ps_misc.tile([128, 512], f32, tag="misc", name="tp")
                    nc.tensor.matmul(
                        tp, r(state["ot"]), r(wo2_sb), start=True, stop=True
                    )
                    out_sb = work.tile([128, 512], bf16, tag="outsb", bufs=4, name="out_sb")
                    nc.vector.tensor_copy(out_sb, tp)
                    st = c * 4 + j
                    nc.sync.dma_start(
                        out=out[st * 128 : (st + 1) * 128, :], in_=out_sb
                    )

                avq = []  # deferred AV/outproj work items for the prev chunk

                def enqueue_chunk_av(c, pts):
                    for j in range(4):
                        state = {}
                        for h in range(2):
                            avq.append(
                                lambda j=j, h=h, pts=pts, state=state: emit_av_group(
                                    j, h, pts, state
                                )
                            )
                        avq.append(
                            lambda c=c, j=j, state=state: emit_outproj(c, j, state)
                        )

                pt_carry = None  # exp output for (c, t=0) computed in chunk c-1
                kv_states = {}

                def kvw(kc):
                    cw = min(512, skc - kc * 512)
                    st = kv_states.setdefault(kc, {})
                    return [
                        lambda: kv_kproj(kc, 0, cw),
                        lambda: kv_vproj(kc, 0, cw, st),
                        lambda: kv_vaug(kc, 0, cw, st, 0, 2),
                        lambda: kv_vaug(kc, 0, cw, st, 2, 4),
                    ]

                # K projections must land in chunk 0 (its own scores consume
                # every key tile), but the last kv chunk's V-side work is
                # first read by AV(chunk 0), which runs during chunk 1 --
                # defer it there so chunk 0's PE keeps pace with ScalarE.
                prework = []
                deferred = []
                if NKC == 4:
                    kp1, vp1, va1a, va1b = kvw(1)
                    kp2, vp2, va2a, va2b = kvw(2)
                    kp3, vp3, va3a, va3b = kvw(3)
                    prework = [kp1, vp1, va1a, va1b, kp2, None, vp2, None,
                               kp3, va2a, va2b]
                    deferred = [vp3, va3a, va3b]
                else:
                    for kc in range(1, NKC):
                        prework.extend(kvw(kc))
                for c in range(NQC):
                    pts = []
                    for t in range(NT):
                        if t == 0 and pt_carry is not None:
                            pt = pt_carry
                            pt_carry = None
                        else:
                            pt = emit_scores_exp(c, t)
                        pts.append(pt)
                        if c == 0 and t == 0 and skc > ksplit:
                            emit_kv(0, lo=0, hi=min(512, skc))
                        if c == 0 and prework and t >= 1:
                            item = prework.pop(0)
                            if item is not None:
                                item()
                        if t == NT // 2 and c + 1 < NQC:
                            emit_qproj(c + 1)
                        if t == NT - 1 and c + 1 < NQC:
                            pt_carry = emit_scores_exp(c + 1, 0)
                        # drain one deferred kv / AV / outproj item per slot
                        if c >= 1 and deferred:
                            deferred.pop(0)()
                        elif avq:
                            avq.pop(0)()
                    # any leftovers (short NT) before enqueueing the new chunk
                    while avq:
                        avq.pop(0)()
                    enqueue_chunk_av(c, pts)
                while avq:
                    avq.pop(0)()

    nc.compile()
    return nc


def _get_runtime(skc: int, reps: int = 1):
    key = (skc, reps)
    if key not in _RUNTIMES:
        _RUNTIMES[key] = _build_program(skc, reps)
    return _RUNTIMES[key]


def _numpy_reference(x1, x2, mask, Wq, bq, Wk, bk, Wv, bv, Wo, bo):
    q = (x1 @ Wq + bq).reshape(B, S, H, DH).transpose(0, 2, 1, 3)
    k = (x2 @ Wk + bk).reshape(B, S, H, DH).transpose(0, 2, 1, 3)
    v = (x2 @ Wv + bv).reshape(B, S, H, DH).transpose(0, 2, 1, 3)
    scores = np.einsum("bhqd,bhkd->bhqk", q, k) / np.sqrt(np.float32(DH))
    scores = scores + mask[:, None, None, :].astype(np.float32) * np.float32(-1e9)
    scores = scores - scores.max(axis=-1, keepdims=True)
    e = np.exp(scores)
    attn = e / e.sum(axis=-1, keepdims=True)
    o = np.einsum("bhqk,bhkd->bhqd", attn, v)
    o = o.transpose(0, 2, 1, 3).reshape(B, S, D)
    return (o @ Wo + bo).astype(np.float32)


def _make_in_maps(x1, x2, mask, Wq, Wk, Wv, Wo):
    import ml_dtypes

    bf16 = ml_dtypes.bfloat16
    keep = [np.nonzero(mask[b] == 0)[0] for b in range(B)]
    counts = [len(k) for k in keep]
    skc = ((max(counts) + 127) // 128) * 128
    nt = skc // 128
    in_maps = []
    for c in range(NCORES):
        b, hp = c // 4, c % 4
        x2c = np.zeros((skc, D), dtype=np.float32)
        x2c[: counts[b]] = x2[b][keep[b]]
        mf = np.zeros((nt, 128), dtype=np.float32)
        mf.reshape(-1)[: counts[b]] = 1.0
        cols = slice(hp * 128, (hp + 1) * 128)
        in_maps.append(
            {
                "x1t": np.ascontiguousarray(x1[b].T).astype(bf16),
                "x2ct": np.ascontiguousarray(x2c.T).astype(bf16),
                "maskb": np.ascontiguousarray(mf.T).astype(bf16),
                "wq": np.ascontiguousarray(Wq[:, cols]).astype(bf16),
                "wk": np.ascontiguousarray(Wk[:, cols]).astype(bf16),
                "wv": np.ascontiguousarray(Wv[:, cols]).astype(bf16),
                "wo2": np.ascontiguousarray(Wo[hp * 128 : (hp + 1) * 128, :]),
            }
        )
    return skc, in_maps


def kernel(x1, x2, mask, Wq, bq, Wk, bk, Wv, bv, Wo, bo):
    from concourse.bass_utils import run_bass_kernel_spmd

    x1 = np.asarray(x1, dtype=np.float32)
    x2 = np.asarray(x2, dtype=np.float32)
    mask = np.asarray(mask)
    Wq = np.asarray(Wq, dtype=np.float32)
    Wk = np.asarray(Wk, dtype=np.float32)
    Wv = np.asarray(Wv, dtype=np.float32)
    Wo = np.asarray(Wo, dtype=np.float32)
    bq, bk, bv, bo = (np.asarray(b, dtype=np.float32) for b in (bq, bk, bv, bo))

    counts = [int((mask[b] == 0).sum()) for b in range(B)]
    if any(np.abs(b).max() > 0 for b in (bq, bk, bv) if b.size) or min(counts) == 0:
        return _numpy_reference(x1, x2, mask, Wq, bq, Wk, bk, Wv, bv, Wo, bo)

    skc, in_maps = _make_in_maps(x1, x2, mask, Wq, Wk, Wv, Wo)
    nc = _get_runtime(skc)

    res = run_bass_kernel_spmd(nc, in_maps, core_ids=list(range(NCORES)))
    full = np.empty((B, S, D), dtype=np.float32)
    for b in range(B):
        acc = res.results[4 * b]["out"].astype(np.float32)
        for hp in range(1, 4):
            acc = acc + res.results[4 * b + hp]["out"].astype(np.float32)
        full[b] = acc + bo
    return full
